# revision 9
# baseline (speedup 1.0000x reference)
"""ContrastHead KNN-contrastive loss on 8 Trainium2 NeuronCores.

Strategy (v4, gather-free):
  - Points sharded 8 ways (12500/core); each core handles the 437500
    (m, k) requests of its own points. dma_gather is NOT used: its Q7
    descriptor generation is dispatch-serialized at ~2.5us per 1024
    indices, capping any gather-based kernel at ~1.1 ms.
  - Instead, requests are sorted by neighbor row j. Each group of <=512
    slots then references a window of <=128 consecutive table rows. The
    host bakes (a) the window rows (bf16 slices of the table), (b) a
    one-hot fp8 expansion matrix E[w, s] (pure index metadata), and
    (c) the transposed fp16 point rows p1t[c, s].
  - Device per group: gT = win^T @ E on TensorE (the "gather"),
    prod = gT * p1t on DVE, then a ones-block matmul reduces prod to 8
    partial sums per slot, packed 16 groups per PSUM bank and written
    out once per 16 groups via the Scalar HWDGE queue (keeping the Sync
    queue free for input prefetch).
  - Host: d2 = |g|^2 + |p|^2 - 2*sum(partials) via norm table lookups,
    then the cheap softmax / masking / reduction.

kernel(**inputs) takes FULL inputs and returns the FULL (scalar) output.
"""
import numpy as np

M_TOTAL = 100000
C = 64
K = 35
N_CORES = 8
M_CORE = M_TOTAL // N_CORES          # 12500
REQ_CORE = M_CORE * K                # 437500 requests per core
S = 512                              # slots per group
WMAX = 128                           # max table-window rows per group
N_G4 = 928                           # padded group count (58 * 16)
OBG = 4                              # groups per output pack (PE col-tiles at 32*r)
LB = 4                               # groups per batched input load

_EPS = 1e-7
TEMPERATURE = 0.1
WEIGHT = 1.0

_cached = {}


def _get_nc():
    if "nc" in _cached:
        return _cached["nc"]
    import concourse.bacc as bacc
    import concourse.mybir as mybir
    import concourse.tile as tile
    import bass_rust
    from concourse.vector_clock import ScopedClock

    # --- walrus in this container rejects >1 sync-wait per instruction. ---
    def _patched_drain_and_barrier(self, tick_clock, wait_clock):
        holder = self.nc.sync.nop(nofuse=True, hint="tile_exit_waits")
        wait_clock.add_sem_waits(
            holder.ins, ScopedClock({None: tick_clock.global_clock})
        )
        si = holder.ins.sync_info
        waits = list(si.on_wait) if si is not None else []
        if len(waits) > 1:
            si.on_wait[:] = waits[:1]
            for w in waits[1:]:
                nop = self.nc.sync.nop(nofuse=True, hint="tile_exit_waits")
                nop.ins.sync_info = mybir.SyncInfo(on_wait=[w], on_update=[])
        self.nc.sync.drain()
        self.nc.all_engine_barrier()
        assert self.sems is not None
        popped = self.nc._tile_sem_poison_stack.pop()
        assert popped is self._sem_poison
        self.nc.clear_and_free_semaphores(list(self.sems.allocated().values()))
        self.nc.all_engine_barrier()

    tile.TileContext._drain_and_barrier = _patched_drain_and_barrier

    def _split_multi_waits(nc, limit=1):
        counter = [0]
        for func in nc.m.functions:
            for bb in func.blocks:
                out = []
                changed = False
                for inst in bb.instructions:
                    si = inst.sync_info
                    waits = list(si.on_wait) if si is not None else []
                    if len(waits) > limit:
                        for w in waits[:-limit]:
                            nop = bass_rust.InstNoOp(
                                name=f"waitsplit-nop-{counter[0]}", ins=[], outs=[]
                            )
                            counter[0] += 1
                            nop.engine = inst.engine
                            nop.sync_info = mybir.SyncInfo(on_wait=[w], on_update=[])
                            nop.bass_nofuse = True
                            out.append(nop)
                        inst.sync_info = mybir.SyncInfo(
                            on_wait=waits[-limit:], on_update=list(si.on_update)
                        )
                        changed = True
                    out.append(inst)
                if changed:
                    bb.instructions = out

    # ---------------------------------------------------------------------
    nc = bacc.Bacc(
        "TRN2", target_bir_lowering=False, debug=False, num_swdge_queues=4
    )
    f32 = mybir.dt.float32
    f16 = mybir.dt.float16
    bf16 = mybir.dt.bfloat16
    f8 = mybir.dt.float8e4

    wins4 = nc.dram_tensor(
        "wins4", [N_G4 // LB, WMAX, LB * C], bf16, kind="ExternalInput"
    )
    e4 = nc.dram_tensor(
        "e4", [N_G4 // LB, WMAX, LB * S], f8, kind="ExternalInput"
    )
    p1t4 = nc.dram_tensor(
        "p1t4", [N_G4 // LB, C, LB * S], f16, kind="ExternalInput"
    )
    ones8 = nc.dram_tensor("ones8", [C, 8], bf16, kind="ExternalInput")
    d2p = nc.dram_tensor(
        "d2p", [N_G4 // OBG, 128, S], f32, kind="ExternalOutput"
    )

    with tile.TileContext(nc) as tc:
        with (
            tc.tile_pool(name="cst", bufs=1) as cst_pool,
            tc.tile_pool(name="ld", bufs=3) as ld_pool,
            tc.tile_pool(name="pr", bufs=4) as pr_pool,
            tc.tile_pool(name="cpx", bufs=2) as cpx_pool,
            tc.tile_pool(name="gtp", bufs=2, space="PSUM") as gtp_pool,
            tc.tile_pool(name="pdp", bufs=2, space="PSUM") as pdp_pool,
        ):
            o8 = cst_pool.tile([C, 8], bf16)
            nc.sync.dma_start(out=o8[:], in_=ones8[:, :])
            w4 = e_t = p_t = prod2 = None
            pds = {}
            pending = []  # (pair_idx, prod2_tile) awaiting lagged mm3

            def emit_mm3(gg, prod_t):
                pk, q = gg // OBG, gg % OBG
                pd = pds[pk]
                nc.tensor.matmul(
                    pd[32 * q : 32 * q + 8, :],
                    o8[:],
                    prod_t[:],
                    start=True,
                    stop=True,
                    tile_position=(0, 32 * q),
                )
                if q == OBG - 1:
                    cp = cpx_pool.tile([128, S], f32)
                    nc.scalar.copy(cp[:], pd[:])
                    nc.scalar.dma_start(out=d2p[pk, :, :], in_=cp[:])

            for g in range(N_G4):
                b, i = g // LB, g % LB
                if i == 0:
                    w4 = ld_pool.tile([WMAX, LB, C], bf16, tag="w")
                    nc.sync.dma_start(
                        out=w4[:].rearrange("p a b -> p (a b)"),
                        in_=wins4[b, :, :],
                    )
                    e_t = ld_pool.tile([WMAX, LB * S], f8, tag="e")
                    nc.sync.dma_start(out=e_t[:], in_=e4[b, :, :])
                    p_t = ld_pool.tile([C, LB * S], f16, tag="p")
                    nc.sync.dma_start(out=p_t[:], in_=p1t4[b, :, :])
                if g % OBG == 0:
                    pds[g // OBG] = pdp_pool.tile([128, S], f32, name='pd')
                prod2 = pr_pool.tile([C, S], bf16, tag="prod")
                gt = gtp_pool.tile([C, S], f32)
                nc.tensor.matmul(
                    gt[:],
                    w4[:, i, :],
                    e_t[:, i * S : (i + 1) * S],
                    start=True,
                    stop=True,
                )
                nc.vector.tensor_tensor(
                    out=prod2[:],
                    in0=gt[:],
                    in1=p_t[:, i * S : (i + 1) * S],
                    op=mybir.AluOpType.mult,
                )
                pending.append((g, prod2))
                if len(pending) > 2:
                    emit_mm3(*pending.pop(0))
            for pr_ in pending:
                emit_mm3(*pr_)

    nc.compile()
    _split_multi_waits(nc)
    _cached["nc"] = nc
    return nc


def prep_inputs(features, labels, neighbor_idx):
    """Sort each core's requests by neighbor row, cut (<=512 slot,
    <=128 row) groups, bake window/one-hot/point streams.
    Returns (in_maps, aux) where aux drives the host-side unmap."""
    import ml_dtypes

    features = np.ascontiguousarray(np.asarray(features), dtype=np.float32)
    tbl16 = features.astype(ml_dtypes.bfloat16)
    tbl16_pad = np.zeros((M_TOTAL + WMAX, C), ml_dtypes.bfloat16)
    tbl16_pad[:M_TOTAL] = tbl16
    feat16 = features.astype(np.float16)
    n16 = (tbl16.astype(np.float32) ** 2).sum(1)          # |bf16 g|^2
    neighbor_idx = np.asarray(neighbor_idx).astype(np.int64)

    ones8 = np.zeros((C, 8), ml_dtypes.bfloat16)
    for i in range(8):
        ones8[8 * i : 8 * i + 8, i] = 1.0

    in_maps = []
    aux = []
    for c in range(N_CORES):
        m0 = c * M_CORE
        nb = neighbor_idx[m0 : m0 + M_CORE]              # [12500, 35]
        j = nb.ravel()                                   # request r = m*35+k
        order = np.argsort(j, kind="stable")
        js = j[order]
        m_s = order // K                                 # local point id

        s0s, es, j0s = [], [], []
        s0 = 0
        while s0 < REQ_CORE:
            j0 = int(js[s0])
            e = min(
                s0 + S,
                int(np.searchsorted(js, j0 + WMAX, "left")),
                REQ_CORE,
            )
            s0s.append(s0)
            es.append(e)
            j0s.append(j0)
            s0 = e
        nG = len(s0s)
        assert nG <= N_G4, f"group overflow: {nG}"
        s0s = np.array(s0s)
        es = np.array(es)
        j0s_pad = np.zeros(N_G4, np.int64)
        j0s_pad[:nG] = np.array(j0s)
        lens = es - s0s

        gid = np.repeat(np.arange(nG), lens)             # [R]
        col = np.arange(REQ_CORE) - np.repeat(s0s, lens)
        jrel = js - np.repeat(j0s_pad[:nG], lens)

        E = np.zeros((N_G4, WMAX, S), ml_dtypes.float8_e4m3fn)
        E[gid, jrel, col] = 1.0
        e4 = np.ascontiguousarray(
            E.reshape(N_G4 // LB, LB, WMAX, S)
            .transpose(0, 2, 1, 3)
            .reshape(N_G4 // LB, WMAX, LB * S)
        )

        winrows = j0s_pad[:, None] + np.arange(WMAX)[None, :]
        wins = tbl16_pad[winrows]                        # [N_G4, 128, 64]
        wins4 = np.ascontiguousarray(
            wins.reshape(N_G4 // LB, LB, WMAX, C)
            .transpose(0, 2, 1, 3)
            .reshape(N_G4 // LB, WMAX, LB * C)
        )

        mpad = np.zeros((N_G4, S), np.int64)
        mpad[gid, col] = m_s
        P = feat16[m0 + mpad]                            # [N_G4, 512, 64]
        p1t4 = np.ascontiguousarray(
            P.transpose(0, 2, 1)
            .reshape(N_G4 // LB, LB, C, S)
            .transpose(0, 2, 1, 3)
            .reshape(N_G4 // LB, C, LB * S)
        )

        jpad = np.zeros((N_G4, S), np.int64)
        jpad[gid, col] = js
        valid = np.zeros((N_G4, S), bool)
        valid[gid, col] = True

        in_maps.append(
            {"wins4": wins4, "e4": e4, "p1t4": p1t4, "ones8": ones8}
        )
        aux.append((order, jpad, mpad, valid))
    _cached["n16"] = n16
    return in_maps, aux


def kernel(features, labels, neighbor_idx):
    from concourse.bass_utils import run_bass_kernel_spmd

    features = np.ascontiguousarray(np.asarray(features), dtype=np.float32)
    labels = np.asarray(labels).astype(np.int64)
    neighbor_idx = np.asarray(neighbor_idx).astype(np.int64)

    nc = _get_nc()
    in_maps, aux = prep_inputs(features, labels, neighbor_idx)
    _cached["in_maps"] = in_maps

    res = run_bass_kernel_spmd(nc, in_maps, list(range(N_CORES))).results

    # ---- host: d2 from norms + device dot products, then reduction ----
    n16 = _cached["n16"]
    feat16 = features.astype(np.float16)
    posmask = (labels[:, None] == labels[neighbor_idx]).astype(np.float32)
    cnt = posmask.sum(-1)
    pm = ((cnt > 0) & (cnt < K)).astype(np.float32)

    loss_num = 0.0
    for c in range(N_CORES):
        m0 = c * M_CORE
        psq = (feat16[m0 : m0 + M_CORE].astype(np.float32) ** 2).sum(1)
        order, jpad, mpad, valid = aux[c]
        gp = (
            res[c]["d2p"]
            .reshape(N_G4 // OBG, OBG, 32, S)[:, :, :8, :]
            .sum(axis=2)
            .reshape(N_G4, S)
        )
        d2_pad = n16[jpad] + psq[mpad] - 2.0 * gp
        d2_grid = np.empty(REQ_CORE, np.float32)
        d2_grid[order] = d2_pad[valid]
        d2_grid = d2_grid.reshape(M_CORE, K)

        dist = np.sqrt(np.maximum(d2_grid, 0.0) + _EPS)
        d = -dist
        d = d - d.max(axis=-1, keepdims=True)
        d = d / TEMPERATURE
        ex = np.exp(d)
        pos = (ex * posmask[m0 : m0 + M_CORE]).sum(-1)
        neg = ex.sum(-1)
        loss = -np.log(pos / neg + _EPS)
        loss_num += float((loss * pm[m0 : m0 + M_CORE]).sum())

    denom = max(float(pm.sum()), 1.0)
    return np.float32(loss_num / denom * WEIGHT)


# revision 10
# speedup vs baseline: 1.1987x; 1.1987x over previous
"""ContrastHead KNN-contrastive loss on 8 Trainium2 NeuronCores.

Strategy (v4, gather-free):
  - Points sharded 8 ways (12500/core); each core handles the 437500
    (m, k) requests of its own points. dma_gather is NOT used: its Q7
    descriptor generation is dispatch-serialized at ~2.5us per 1024
    indices, capping any gather-based kernel at ~1.1 ms.
  - Instead, requests are sorted by neighbor row j. Each group of <=512
    slots then references a window of <=128 consecutive table rows. The
    host bakes (a) the window rows (bf16 slices of the table), (b) a
    one-hot fp8 expansion matrix E[w, s] (pure index metadata), and
    (c) the transposed fp16 point rows p1t[c, s].
  - Device per group: gT = win^T @ E on TensorE (the "gather"),
    prod = gT * p1t on DVE, then a ones-block matmul reduces prod to 8
    partial sums per slot, packed 16 groups per PSUM bank and written
    out once per 16 groups via the Scalar HWDGE queue (keeping the Sync
    queue free for input prefetch).
  - Host: d2 = |g|^2 + |p|^2 - 2*sum(partials) via norm table lookups,
    then the cheap softmax / masking / reduction.

kernel(**inputs) takes FULL inputs and returns the FULL (scalar) output.
"""
import numpy as np

M_TOTAL = 100000
C = 64
K = 35
N_CORES = 8
M_CORE = M_TOTAL // N_CORES          # 12500
REQ_CORE = M_CORE * K                # 437500 requests per core
S = 512                              # slots per group
WMAX = 128                           # max table-window rows per group
N_G4 = 928                           # padded group count (58 * 16)
OBG = 4                              # groups per output pack (PE col-tiles at 32*r)
LB = 4                               # groups per batched input load

_EPS = 1e-7
TEMPERATURE = 0.1
WEIGHT = 1.0

_cached = {}


def _get_nc():
    if "nc" in _cached:
        return _cached["nc"]
    import concourse.bacc as bacc
    import concourse.mybir as mybir
    import concourse.tile as tile
    import bass_rust
    from concourse.vector_clock import ScopedClock

    # --- walrus in this container rejects >1 sync-wait per instruction. ---
    def _patched_drain_and_barrier(self, tick_clock, wait_clock):
        holder = self.nc.sync.nop(nofuse=True, hint="tile_exit_waits")
        wait_clock.add_sem_waits(
            holder.ins, ScopedClock({None: tick_clock.global_clock})
        )
        si = holder.ins.sync_info
        waits = list(si.on_wait) if si is not None else []
        if len(waits) > 1:
            si.on_wait[:] = waits[:1]
            for w in waits[1:]:
                nop = self.nc.sync.nop(nofuse=True, hint="tile_exit_waits")
                nop.ins.sync_info = mybir.SyncInfo(on_wait=[w], on_update=[])
        self.nc.sync.drain()
        self.nc.all_engine_barrier()
        assert self.sems is not None
        popped = self.nc._tile_sem_poison_stack.pop()
        assert popped is self._sem_poison
        self.nc.clear_and_free_semaphores(list(self.sems.allocated().values()))
        self.nc.all_engine_barrier()

    tile.TileContext._drain_and_barrier = _patched_drain_and_barrier

    def _split_multi_waits(nc, limit=1):
        counter = [0]
        for func in nc.m.functions:
            for bb in func.blocks:
                out = []
                changed = False
                for inst in bb.instructions:
                    si = inst.sync_info
                    waits = list(si.on_wait) if si is not None else []
                    if len(waits) > limit:
                        for w in waits[:-limit]:
                            nop = bass_rust.InstNoOp(
                                name=f"waitsplit-nop-{counter[0]}", ins=[], outs=[]
                            )
                            counter[0] += 1
                            nop.engine = inst.engine
                            nop.sync_info = mybir.SyncInfo(on_wait=[w], on_update=[])
                            nop.bass_nofuse = True
                            out.append(nop)
                        inst.sync_info = mybir.SyncInfo(
                            on_wait=waits[-limit:], on_update=list(si.on_update)
                        )
                        changed = True
                    out.append(inst)
                if changed:
                    bb.instructions = out

    # ---------------------------------------------------------------------
    nc = bacc.Bacc(
        "TRN2", target_bir_lowering=False, debug=False, num_swdge_queues=4
    )
    f32 = mybir.dt.float32
    f16 = mybir.dt.float16
    bf16 = mybir.dt.bfloat16
    f8 = mybir.dt.float8e4

    wins4 = nc.dram_tensor(
        "wins4", [N_G4 // LB, WMAX, LB * C], bf16, kind="ExternalInput"
    )
    e4 = nc.dram_tensor(
        "e4", [N_G4 // LB, WMAX, LB * S], f8, kind="ExternalInput"
    )
    p1t4 = nc.dram_tensor(
        "p1t4", [N_G4 // LB, C, LB * S], f16, kind="ExternalInput"
    )
    ones8 = nc.dram_tensor("ones8", [C, 8], bf16, kind="ExternalInput")
    d2p = nc.dram_tensor(
        "d2p", [N_G4 // OBG, 128, S], f32, kind="ExternalOutput"
    )

    with tile.TileContext(nc) as tc:
        with (
            tc.tile_pool(name="cst", bufs=1) as cst_pool,
            tc.tile_pool(name="ld", bufs=3) as ld_pool,
            tc.tile_pool(name="pr", bufs=4) as pr_pool,
            tc.tile_pool(name="cpx", bufs=2) as cpx_pool,
            tc.tile_pool(name="gtp", bufs=2, space="PSUM") as gtp_pool,
            tc.tile_pool(name="pdp", bufs=2, space="PSUM") as pdp_pool,
        ):
            o8 = cst_pool.tile([C, 8], bf16)
            nc.sync.dma_start(out=o8[:], in_=ones8[:, :])
            w4 = e_t = p_t = pd = None
            for g in range(N_G4):
                b, i = g // LB, g % LB
                if i == 0:
                    w4 = ld_pool.tile([WMAX, LB, C], bf16, tag="w")
                    nc.sync.dma_start(
                        out=w4[:].rearrange("p a b -> p (a b)"),
                        in_=wins4[b, :, :],
                    )
                    e_t = ld_pool.tile([WMAX, LB * S], f8, tag="e")
                    nc.sync.dma_start(out=e_t[:], in_=e4[b, :, :])
                    p_t = ld_pool.tile([C, LB * S], f16, tag="p")
                    nc.sync.dma_start(out=p_t[:], in_=p1t4[b, :, :])
                if g % OBG == 0:
                    pd = pdp_pool.tile([128, S], f32)
                gt = gtp_pool.tile([C, S], f32)
                nc.tensor.matmul(
                    gt[:],
                    w4[:, i, :],
                    e_t[:, i * S : (i + 1) * S],
                    start=True,
                    stop=True,
                )
                prod = pr_pool.tile([C, S], bf16)
                nc.vector.tensor_tensor(
                    out=prod[:],
                    in0=gt[:],
                    in1=p_t[:, i * S : (i + 1) * S],
                    op=mybir.AluOpType.mult,
                )
                r = g % OBG
                nc.tensor.matmul(
                    pd[32 * r : 32 * r + 8, :],
                    o8[:],
                    prod[:],
                    start=True,
                    stop=True,
                    tile_position=(0, 32 * r),
                )
                if r == OBG - 1:
                    cp = cpx_pool.tile([128, S], f32)
                    nc.vector.tensor_copy(cp[:], pd[:])
                    nc.scalar.dma_start(out=d2p[g // OBG, :, :], in_=cp[:])

    nc.compile()
    _split_multi_waits(nc)
    _cached["nc"] = nc
    return nc


def prep_inputs(features, labels, neighbor_idx):
    """Sort each core's requests by neighbor row, cut (<=512 slot,
    <=128 row) groups, bake window/one-hot/point streams.
    Returns (in_maps, aux) where aux drives the host-side unmap."""
    import ml_dtypes

    features = np.ascontiguousarray(np.asarray(features), dtype=np.float32)
    tbl16 = features.astype(ml_dtypes.bfloat16)
    tbl16_pad = np.zeros((M_TOTAL + WMAX, C), ml_dtypes.bfloat16)
    tbl16_pad[:M_TOTAL] = tbl16
    feat16 = features.astype(np.float16)
    n16 = (tbl16.astype(np.float32) ** 2).sum(1)          # |bf16 g|^2
    neighbor_idx = np.asarray(neighbor_idx).astype(np.int64)

    ones8 = np.zeros((C, 8), ml_dtypes.bfloat16)
    for i in range(8):
        ones8[8 * i : 8 * i + 8, i] = 1.0

    in_maps = []
    aux = []
    for c in range(N_CORES):
        m0 = c * M_CORE
        nb = neighbor_idx[m0 : m0 + M_CORE]              # [12500, 35]
        j = nb.ravel()                                   # request r = m*35+k
        order = np.argsort(j, kind="stable")
        js = j[order]
        m_s = order // K                                 # local point id

        s0s, es, j0s = [], [], []
        s0 = 0
        while s0 < REQ_CORE:
            j0 = int(js[s0])
            e = min(
                s0 + S,
                int(np.searchsorted(js, j0 + WMAX, "left")),
                REQ_CORE,
            )
            s0s.append(s0)
            es.append(e)
            j0s.append(j0)
            s0 = e
        nG = len(s0s)
        assert nG <= N_G4, f"group overflow: {nG}"
        s0s = np.array(s0s)
        es = np.array(es)
        j0s_pad = np.zeros(N_G4, np.int64)
        j0s_pad[:nG] = np.array(j0s)
        lens = es - s0s

        gid = np.repeat(np.arange(nG), lens)             # [R]
        col = np.arange(REQ_CORE) - np.repeat(s0s, lens)
        jrel = js - np.repeat(j0s_pad[:nG], lens)

        E = np.zeros((N_G4, WMAX, S), ml_dtypes.float8_e4m3fn)
        E[gid, jrel, col] = 1.0
        e4 = np.ascontiguousarray(
            E.reshape(N_G4 // LB, LB, WMAX, S)
            .transpose(0, 2, 1, 3)
            .reshape(N_G4 // LB, WMAX, LB * S)
        )

        winrows = j0s_pad[:, None] + np.arange(WMAX)[None, :]
        wins = tbl16_pad[winrows]                        # [N_G4, 128, 64]
        wins4 = np.ascontiguousarray(
            wins.reshape(N_G4 // LB, LB, WMAX, C)
            .transpose(0, 2, 1, 3)
            .reshape(N_G4 // LB, WMAX, LB * C)
        )

        mpad = np.zeros((N_G4, S), np.int64)
        mpad[gid, col] = m_s
        P = feat16[m0 + mpad]                            # [N_G4, 512, 64]
        p1t4 = np.ascontiguousarray(
            P.transpose(0, 2, 1)
            .reshape(N_G4 // LB, LB, C, S)
            .transpose(0, 2, 1, 3)
            .reshape(N_G4 // LB, C, LB * S)
        )

        jpad = np.zeros((N_G4, S), np.int64)
        jpad[gid, col] = js
        valid = np.zeros((N_G4, S), bool)
        valid[gid, col] = True

        in_maps.append(
            {"wins4": wins4, "e4": e4, "p1t4": p1t4, "ones8": ones8}
        )
        aux.append((order, jpad, mpad, valid))
    _cached["n16"] = n16
    return in_maps, aux


def kernel(features, labels, neighbor_idx):
    from concourse.bass_utils import run_bass_kernel_spmd

    features = np.ascontiguousarray(np.asarray(features), dtype=np.float32)
    labels = np.asarray(labels).astype(np.int64)
    neighbor_idx = np.asarray(neighbor_idx).astype(np.int64)

    nc = _get_nc()
    in_maps, aux = prep_inputs(features, labels, neighbor_idx)
    _cached["in_maps"] = in_maps

    res = run_bass_kernel_spmd(nc, in_maps, list(range(N_CORES))).results

    # ---- host: d2 from norms + device dot products, then reduction ----
    n16 = _cached["n16"]
    feat16 = features.astype(np.float16)
    posmask = (labels[:, None] == labels[neighbor_idx]).astype(np.float32)
    cnt = posmask.sum(-1)
    pm = ((cnt > 0) & (cnt < K)).astype(np.float32)

    loss_num = 0.0
    for c in range(N_CORES):
        m0 = c * M_CORE
        psq = (feat16[m0 : m0 + M_CORE].astype(np.float32) ** 2).sum(1)
        order, jpad, mpad, valid = aux[c]
        gp = (
            res[c]["d2p"]
            .reshape(N_G4 // OBG, OBG, 32, S)[:, :, :8, :]
            .sum(axis=2)
            .reshape(N_G4, S)
        )
        d2_pad = n16[jpad] + psq[mpad] - 2.0 * gp
        d2_grid = np.empty(REQ_CORE, np.float32)
        d2_grid[order] = d2_pad[valid]
        d2_grid = d2_grid.reshape(M_CORE, K)

        dist = np.sqrt(np.maximum(d2_grid, 0.0) + _EPS)
        d = -dist
        d = d - d.max(axis=-1, keepdims=True)
        d = d / TEMPERATURE
        ex = np.exp(d)
        pos = (ex * posmask[m0 : m0 + M_CORE]).sum(-1)
        neg = ex.sum(-1)
        loss = -np.log(pos / neg + _EPS)
        loss_num += float((loss * pm[m0 : m0 + M_CORE]).sum())

    denom = max(float(pm.sum()), 1.0)
    return np.float32(loss_num / denom * WEIGHT)


# revision 11
# speedup vs baseline: 1.1991x; 1.0003x over previous
"""ContrastHead KNN-contrastive loss on 8 Trainium2 NeuronCores.

Strategy (v4, gather-free):
  - Points sharded 8 ways (12500/core); each core handles the 437500
    (m, k) requests of its own points. dma_gather is NOT used: its Q7
    descriptor generation is dispatch-serialized at ~2.5us per 1024
    indices, capping any gather-based kernel at ~1.1 ms.
  - Instead, requests are sorted by neighbor row j. Each group of <=512
    slots then references a window of <=128 consecutive table rows. The
    host bakes (a) the window rows (bf16 slices of the table), (b) a
    one-hot fp8 expansion matrix E[w, s] (pure index metadata), and
    (c) the transposed fp16 point rows p1t[c, s].
  - Device per group: gT = win^T @ E on TensorE (the "gather"),
    prod = gT * p1t on DVE, then a ones-block matmul reduces prod to 8
    partial sums per slot, packed 16 groups per PSUM bank and written
    out once per 16 groups via the Scalar HWDGE queue (keeping the Sync
    queue free for input prefetch).
  - Host: d2 = |g|^2 + |p|^2 - 2*sum(partials) via norm table lookups,
    then the cheap softmax / masking / reduction.

kernel(**inputs) takes FULL inputs and returns the FULL (scalar) output.
"""
import numpy as np

M_TOTAL = 100000
C = 64
K = 35
N_CORES = 8
M_CORE = M_TOTAL // N_CORES          # 12500
REQ_CORE = M_CORE * K                # 437500 requests per core
S = 512                              # slots per group
WMAX = 128                           # max table-window rows per group
N_G4 = 928                           # padded group count (58 * 16)
OBG = 4                              # groups per output pack (PE col-tiles at 32*r)
LB = 4                               # groups per batched input load

_EPS = 1e-7
TEMPERATURE = 0.1
WEIGHT = 1.0

_cached = {}


def _get_nc():
    if "nc" in _cached:
        return _cached["nc"]
    import concourse.bacc as bacc
    import concourse.mybir as mybir
    import concourse.tile as tile
    import bass_rust
    from concourse.vector_clock import ScopedClock

    # --- walrus in this container rejects >1 sync-wait per instruction. ---
    def _patched_drain_and_barrier(self, tick_clock, wait_clock):
        holder = self.nc.sync.nop(nofuse=True, hint="tile_exit_waits")
        wait_clock.add_sem_waits(
            holder.ins, ScopedClock({None: tick_clock.global_clock})
        )
        si = holder.ins.sync_info
        waits = list(si.on_wait) if si is not None else []
        if len(waits) > 1:
            si.on_wait[:] = waits[:1]
            for w in waits[1:]:
                nop = self.nc.sync.nop(nofuse=True, hint="tile_exit_waits")
                nop.ins.sync_info = mybir.SyncInfo(on_wait=[w], on_update=[])
        self.nc.sync.drain()
        self.nc.all_engine_barrier()
        assert self.sems is not None
        popped = self.nc._tile_sem_poison_stack.pop()
        assert popped is self._sem_poison
        self.nc.clear_and_free_semaphores(list(self.sems.allocated().values()))
        self.nc.all_engine_barrier()

    tile.TileContext._drain_and_barrier = _patched_drain_and_barrier

    def _split_multi_waits(nc, limit=1):
        counter = [0]
        for func in nc.m.functions:
            for bb in func.blocks:
                out = []
                changed = False
                for inst in bb.instructions:
                    si = inst.sync_info
                    waits = list(si.on_wait) if si is not None else []
                    if len(waits) > limit:
                        for w in waits[:-limit]:
                            nop = bass_rust.InstNoOp(
                                name=f"waitsplit-nop-{counter[0]}", ins=[], outs=[]
                            )
                            counter[0] += 1
                            nop.engine = inst.engine
                            nop.sync_info = mybir.SyncInfo(on_wait=[w], on_update=[])
                            nop.bass_nofuse = True
                            out.append(nop)
                        inst.sync_info = mybir.SyncInfo(
                            on_wait=waits[-limit:], on_update=list(si.on_update)
                        )
                        changed = True
                    out.append(inst)
                if changed:
                    bb.instructions = out

    # ---------------------------------------------------------------------
    nc = bacc.Bacc(
        "TRN2", target_bir_lowering=False, debug=False, num_swdge_queues=4
    )
    f32 = mybir.dt.float32
    f16 = mybir.dt.float16
    bf16 = mybir.dt.bfloat16
    f8 = mybir.dt.float8e4

    wins4 = nc.dram_tensor(
        "wins4", [N_G4 // LB, WMAX, LB * C], bf16, kind="ExternalInput"
    )
    e4 = nc.dram_tensor(
        "e4", [N_G4 // LB, WMAX, LB * S], f8, kind="ExternalInput"
    )
    p1t4 = nc.dram_tensor(
        "p1t4", [N_G4 // LB, C, LB * S], f16, kind="ExternalInput"
    )
    ones8 = nc.dram_tensor("ones8", [C, 8], bf16, kind="ExternalInput")
    d2p = nc.dram_tensor(
        "d2p", [N_G4 // OBG, 128, S], f32, kind="ExternalOutput"
    )

    with tile.TileContext(nc) as tc:
        with (
            tc.tile_pool(name="cst", bufs=1) as cst_pool,
            tc.tile_pool(name="ld", bufs=3) as ld_pool,
            tc.tile_pool(name="pr", bufs=6) as pr_pool,
            tc.tile_pool(name="cpx", bufs=2) as cpx_pool,
            tc.tile_pool(name="gtp", bufs=4, space="PSUM") as gtp_pool,
            tc.tile_pool(name="pdp", bufs=3, space="PSUM") as pdp_pool,
        ):
            o8 = cst_pool.tile([C, 8], bf16)
            nc.sync.dma_start(out=o8[:], in_=ones8[:, :])
            w4 = e_t = p_t = pd = None
            for g in range(N_G4):
                b, i = g // LB, g % LB
                if i == 0:
                    w4 = ld_pool.tile([WMAX, LB, C], bf16, tag="w")
                    nc.sync.dma_start(
                        out=w4[:].rearrange("p a b -> p (a b)"),
                        in_=wins4[b, :, :],
                    )
                    e_t = ld_pool.tile([WMAX, LB * S], f8, tag="e")
                    nc.sync.dma_start(out=e_t[:], in_=e4[b, :, :])
                    p_t = ld_pool.tile([C, LB * S], f16, tag="p")
                    nc.sync.dma_start(out=p_t[:], in_=p1t4[b, :, :])
                if g % OBG == 0:
                    pd = pdp_pool.tile([128, S], f32)
                gt = gtp_pool.tile([C, S], f32)
                nc.tensor.matmul(
                    gt[:],
                    w4[:, i, :],
                    e_t[:, i * S : (i + 1) * S],
                    start=True,
                    stop=True,
                )
                prod = pr_pool.tile([C, S], bf16)
                nc.vector.tensor_tensor(
                    out=prod[:],
                    in0=gt[:],
                    in1=p_t[:, i * S : (i + 1) * S],
                    op=mybir.AluOpType.mult,
                )
                r = g % OBG
                nc.tensor.matmul(
                    pd[32 * r : 32 * r + 8, :],
                    o8[:],
                    prod[:],
                    start=True,
                    stop=True,
                    tile_position=(0, 32 * r),
                )
                if r == OBG - 1:
                    cp = cpx_pool.tile([128, S], f32)
                    nc.vector.tensor_copy(cp[:], pd[:])
                    nc.scalar.dma_start(out=d2p[g // OBG, :, :], in_=cp[:])

    nc.compile()
    _split_multi_waits(nc)
    _cached["nc"] = nc
    return nc


def prep_inputs(features, labels, neighbor_idx):
    """Sort each core's requests by neighbor row, cut (<=512 slot,
    <=128 row) groups, bake window/one-hot/point streams.
    Returns (in_maps, aux) where aux drives the host-side unmap."""
    import ml_dtypes

    features = np.ascontiguousarray(np.asarray(features), dtype=np.float32)
    tbl16 = features.astype(ml_dtypes.bfloat16)
    tbl16_pad = np.zeros((M_TOTAL + WMAX, C), ml_dtypes.bfloat16)
    tbl16_pad[:M_TOTAL] = tbl16
    feat16 = features.astype(np.float16)
    n16 = (tbl16.astype(np.float32) ** 2).sum(1)          # |bf16 g|^2
    neighbor_idx = np.asarray(neighbor_idx).astype(np.int64)

    ones8 = np.zeros((C, 8), ml_dtypes.bfloat16)
    for i in range(8):
        ones8[8 * i : 8 * i + 8, i] = 1.0

    in_maps = []
    aux = []
    for c in range(N_CORES):
        m0 = c * M_CORE
        nb = neighbor_idx[m0 : m0 + M_CORE]              # [12500, 35]
        j = nb.ravel()                                   # request r = m*35+k
        order = np.argsort(j, kind="stable")
        js = j[order]
        m_s = order // K                                 # local point id

        s0s, es, j0s = [], [], []
        s0 = 0
        while s0 < REQ_CORE:
            j0 = int(js[s0])
            e = min(
                s0 + S,
                int(np.searchsorted(js, j0 + WMAX, "left")),
                REQ_CORE,
            )
            s0s.append(s0)
            es.append(e)
            j0s.append(j0)
            s0 = e
        nG = len(s0s)
        assert nG <= N_G4, f"group overflow: {nG}"
        s0s = np.array(s0s)
        es = np.array(es)
        j0s_pad = np.zeros(N_G4, np.int64)
        j0s_pad[:nG] = np.array(j0s)
        lens = es - s0s

        gid = np.repeat(np.arange(nG), lens)             # [R]
        col = np.arange(REQ_CORE) - np.repeat(s0s, lens)
        jrel = js - np.repeat(j0s_pad[:nG], lens)

        E = np.zeros((N_G4, WMAX, S), ml_dtypes.float8_e4m3fn)
        E[gid, jrel, col] = 1.0
        e4 = np.ascontiguousarray(
            E.reshape(N_G4 // LB, LB, WMAX, S)
            .transpose(0, 2, 1, 3)
            .reshape(N_G4 // LB, WMAX, LB * S)
        )

        winrows = j0s_pad[:, None] + np.arange(WMAX)[None, :]
        wins = tbl16_pad[winrows]                        # [N_G4, 128, 64]
        wins4 = np.ascontiguousarray(
            wins.reshape(N_G4 // LB, LB, WMAX, C)
            .transpose(0, 2, 1, 3)
            .reshape(N_G4 // LB, WMAX, LB * C)
        )

        mpad = np.zeros((N_G4, S), np.int64)
        mpad[gid, col] = m_s
        P = feat16[m0 + mpad]                            # [N_G4, 512, 64]
        p1t4 = np.ascontiguousarray(
            P.transpose(0, 2, 1)
            .reshape(N_G4 // LB, LB, C, S)
            .transpose(0, 2, 1, 3)
            .reshape(N_G4 // LB, C, LB * S)
        )

        jpad = np.zeros((N_G4, S), np.int64)
        jpad[gid, col] = js
        valid = np.zeros((N_G4, S), bool)
        valid[gid, col] = True

        in_maps.append(
            {"wins4": wins4, "e4": e4, "p1t4": p1t4, "ones8": ones8}
        )
        aux.append((order, jpad, mpad, valid))
    _cached["n16"] = n16
    return in_maps, aux


def kernel(features, labels, neighbor_idx):
    from concourse.bass_utils import run_bass_kernel_spmd

    features = np.ascontiguousarray(np.asarray(features), dtype=np.float32)
    labels = np.asarray(labels).astype(np.int64)
    neighbor_idx = np.asarray(neighbor_idx).astype(np.int64)

    nc = _get_nc()
    in_maps, aux = prep_inputs(features, labels, neighbor_idx)
    _cached["in_maps"] = in_maps

    res = run_bass_kernel_spmd(nc, in_maps, list(range(N_CORES))).results

    # ---- host: d2 from norms + device dot products, then reduction ----
    n16 = _cached["n16"]
    feat16 = features.astype(np.float16)
    posmask = (labels[:, None] == labels[neighbor_idx]).astype(np.float32)
    cnt = posmask.sum(-1)
    pm = ((cnt > 0) & (cnt < K)).astype(np.float32)

    loss_num = 0.0
    for c in range(N_CORES):
        m0 = c * M_CORE
        psq = (feat16[m0 : m0 + M_CORE].astype(np.float32) ** 2).sum(1)
        order, jpad, mpad, valid = aux[c]
        gp = (
            res[c]["d2p"]
            .reshape(N_G4 // OBG, OBG, 32, S)[:, :, :8, :]
            .sum(axis=2)
            .reshape(N_G4, S)
        )
        d2_pad = n16[jpad] + psq[mpad] - 2.0 * gp
        d2_grid = np.empty(REQ_CORE, np.float32)
        d2_grid[order] = d2_pad[valid]
        d2_grid = d2_grid.reshape(M_CORE, K)

        dist = np.sqrt(np.maximum(d2_grid, 0.0) + _EPS)
        d = -dist
        d = d - d.max(axis=-1, keepdims=True)
        d = d / TEMPERATURE
        ex = np.exp(d)
        pos = (ex * posmask[m0 : m0 + M_CORE]).sum(-1)
        neg = ex.sum(-1)
        loss = -np.log(pos / neg + _EPS)
        loss_num += float((loss * pm[m0 : m0 + M_CORE]).sum())

    denom = max(float(pm.sum()), 1.0)
    return np.float32(loss_num / denom * WEIGHT)
